# revision 1
# baseline (speedup 1.0000x reference)
"""Trainium2 Bass kernel for BDH recurrent (chunked linear) attention.

Problem shapes (hardcoded): Q_raw [2,16,2048,256] f32, V_raw [2,2048,1024] f32,
out [2,16,2048,1024] f32.  8 NeuronCores, data+head parallel: each core owns
4 (batch, head) pairs; V is shared across the 4 heads of a core's batch.

Math (reference semantics), per (b,h), chunks of 128:
  QR = rope(Q); KR = QR
  out_c = q_c @ state_{<c} + (q_c q_c^T  * strict_tril) v_c
  state += q_c^T v_c
Implemented with superchunks of SUP chunks: the recurrent state is accumulated
in PSUM (fp32) across superchunks; within a superchunk the chunk-level
causality is handled with explicit per-pair transposed score blocks
G(j,i) = qr_j qr_i^T (which is scores^T, exactly the lhsT layout the
PV matmul needs; the diagonal block gets the transposed strict-tril mask).

RoPE is computed twice, in the natural [t, n] layout (for the state update's
lhsT) and in the transposed [n, t] layout (for the m1/G lhsT) — the host
supplies Q in both layouts (pair-deinterleaved: (evens | odds), so the
rotation is two plane-wise multiply/adds with no interleave shuffles).
All DRAM layouts are partition-major so every DMA is 128 contiguous
descriptors; the output is written partition-major and un-permuted on host.
"""

import numpy as np
import ml_dtypes

import concourse.mybir as mybir
import concourse.tile as tile
from concourse import bacc
from concourse.bass import ds
from concourse.bass_utils import run_bass_kernel_spmd
from concourse.masks import make_identity

B, NH, T, N, D = 2, 16, 2048, 256, 1024
P = 128          # partition / chunk size
NCH = T // P     # 16 chunks
SUP = 4          # chunks per superchunk
NSUP = NCH // SUP
HPC = 4          # (b,h) pairs per core
NCORES = 8
THETA = 2.0 ** 16
TWO_PI = 2.0 * np.pi

bf = mybir.dt.bfloat16
f32 = mybir.dt.float32
bf_np = ml_dtypes.bfloat16

mult = mybir.AluOpType.mult
add_op = mybir.AluOpType.add
sub_op = mybir.AluOpType.subtract

# engine assignment knobs (tuned from profiles).
# NB: gpsimd tensor_tensor contends with DVE's shared SBUF port (measured 4x
# slowdown on BOTH when concurrent) -> keep all tensor_tensor on DVE.
ROPE_ENG_NAT = "vvvvvv"
ROPE_ENG_TR = "vvvvvv"
# state cast engines for the two [128,1024] tiles
STATE_CAST_ENG = ("v", "s")
# out evacuation engine by chunk parity
OUT_EVAC_ENG = ("s", "v")


def _eng(nc, c):
    return {"g": nc.gpsimd, "v": nc.vector, "s": nc.scalar}[c]


def _copy(nc, c, out, in_):
    if c == "s":
        nc.scalar.copy(out, in_)
    else:
        _eng(nc, c).tensor_copy(out, in_)


def _emit_body(nc, tc, qn, qt, v, cn, sn, ct, st, mskT, out):
    """Tile program for one core: 4 (b,h) pairs, full scan each."""
    with (
        tc.tile_pool(name="const", bufs=1) as constp,
        tc.tile_pool(name="qpool", bufs=2) as qpool,
        tc.tile_pool(name="work", bufs=6) as work,
        tc.tile_pool(name="tmppool", bufs=1) as tmpp,
        tc.tile_pool(name="outbuf", bufs=1) as outp,
        tc.tile_pool(name="statesb", bufs=2) as statep,
        tc.tile_pool(name="ps_state", bufs=1, space="PSUM") as ps_state,
        tc.tile_pool(name="ps_out", bufs=2, space="PSUM") as ps_out,
        tc.tile_pool(name="ps_g", bufs=2, space="PSUM") as ps_g,
    ):
        # resident constants (all DRAM layouts partition-major/contiguous).
        # Load order matters for the startup ramp: the transposed-rope
        # tables and first q tiles gate the first matmuls, so they go first;
        # V is split so the first superchunk's slice lands early.
        ct_sb = constp.tile([P, T], bf)
        nc.sync.dma_start(ct_sb[:], ct[:, :])
        st_sb = constp.tile([P, T], bf)
        nc.sync.dma_start(st_sb[:], st[:, :])
        msk_sb = constp.tile([P, SUP * P], bf)
        nc.sync.dma_start(msk_sb[:], mskT[:, :])
        ident = constp.tile([P, P], bf)
        make_identity(nc, ident)
        cn_sb = constp.tile([P, NCH, P], bf)
        nc.sync.dma_start(cn_sb[:], cn[:, :, :])
        sn_sb = constp.tile([P, NCH, P], bf)
        nc.sync.dma_start(sn_sb[:], sn[:, :, :])
        v_sb = constp.tile([P, NCH, D], bf)
        nc.sync.dma_start(v_sb[:, :SUP], v[:, :SUP, :])
        nc.sync.dma_start(v_sb[:, SUP:], v[:, SUP:, :])

        for bh in range(HPC):
            qt_sb = qpool.tile([P, 2, T], bf, tag="qt")
            nc.scalar.dma_start(qt_sb[:, 0], qt[bh, 0])
            nc.scalar.dma_start(qt_sb[:, 1], qt[bh, 1])
            qn_sb = qpool.tile([P, 2, NCH, P], bf, tag="qn")
            nc.scalar.dma_start(qn_sb[:], qn[bh])

            # RoPE, transposed layout [n', t] — emitted FIRST: it gates the
            # G/m1 matmuls, and the DVE queue is strict FIFO.
            qrT = qpool.tile([P, 2, T], bf, tag="qrT")
            tmp2 = tmpp.tile([P, T], bf, tag="ropetmp2")
            tmp3 = tmpp.tile([P, T], bf, tag="ropetmp3")
            e = [_eng(nc, c) for c in ROPE_ENG_TR]
            qte, qto = qt_sb[:, 0], qt_sb[:, 1]
            e[0].tensor_tensor(qrT[:, 0], qte, ct_sb[:], mult)
            e[1].tensor_tensor(tmp2[:], qto, st_sb[:], mult)
            e[2].tensor_tensor(qrT[:, 0], qrT[:, 0], tmp2[:], sub_op)
            e[3].tensor_tensor(qrT[:, 1], qto, ct_sb[:], mult)
            e[4].tensor_tensor(tmp3[:], qte, st_sb[:], mult)
            e[5].tensor_tensor(qrT[:, 1], qrT[:, 1], tmp3[:], add_op)

            # RoPE, natural layout (planes are (evens | odds) over pairs):
            #   qr_e = qe*c - qo*s ;  qr_o = qo*c + qe*s
            # Emitted lazily (after the first superchunk's G evacuations):
            # the DVE queue is strict FIFO and qr only gates the m4 state
            # update, so this keeps the first G/PV matmuls unblocked.
            qr = qpool.tile([P, 2, NCH, P], bf, tag="qr")

            def emit_nat_rope(c0=0):
                tmp0 = tmpp.tile([P, NCH, P], bf, tag="ropetmp0")
                tmp1 = tmpp.tile([P, NCH, P], bf, tag="ropetmp1")
                e = [_eng(nc, c) for c in ROPE_ENG_NAT]
                qe, qo = qn_sb[:, 0, c0:], qn_sb[:, 1, c0:]
                cns, sns = cn_sb[:, c0:], sn_sb[:, c0:]
                q0, q1 = qr[:, 0, c0:], qr[:, 1, c0:]
                t0_, t1_ = tmp0[:, c0:], tmp1[:, c0:]
                e[0].tensor_tensor(q0, qe, cns, mult)
                e[1].tensor_tensor(t0_, qo, sns, mult)
                e[2].tensor_tensor(q0, q0, t0_, sub_op)
                e[3].tensor_tensor(q1, qo, cns, mult)
                e[4].tensor_tensor(t1_, qe, sns, mult)
                e[5].tensor_tensor(q1, q1, t1_, add_op)

            # chunked scan with PSUM-resident state (fp32, 4 banks)
            state_ps = ps_state.tile([P, 2, D], f32, tag="state")
            out_sbs = [
                outp.tile([P, NCH // 2, D], bf, tag=f"out{h}", name=f"out_sb{h}")
                for h in range(2)
            ]
            for s in range(NSUP):
                if s > 0:
                    state_sb = statep.tile([P, 2, D], bf, tag="state_sb")
                    for m in range(2):
                        for h in range(2):
                            dsl = ds(h * 512, 512)
                            _copy(
                                nc, STATE_CAST_ENG[h],
                                state_sb[:, m, dsl], state_ps[:, m, dsl],
                            )

                # Batched transposed score blocks: for each j-chunk of the
                # superchunk, G_j = qr_j^T-contraction against all i >= j in
                # one matmul (N spans the remaining chunks).  The combined
                # mask (strict-triu block then ones) masks the diagonal
                # block in the same evacuation op.
                g_sbs = []
                for cj in range(SUP):
                    j = s * SUP + cj
                    w = (SUP - cj) * P
                    g_ps = ps_g.tile([P, 512], f32, tag="g", name="g_ps")
                    nc.tensor.matmul(
                        g_ps[:, :w], qrT[:, 0, ds(j * P, P)],
                        qrT[:, 0, ds(j * P, w)], start=True, stop=False,
                    )
                    nc.tensor.matmul(
                        g_ps[:, :w], qrT[:, 1, ds(j * P, P)],
                        qrT[:, 1, ds(j * P, w)], start=False, stop=True,
                    )
                    g_sb = work.tile([P, 512], bf, tag="gsb", name="g_sb")
                    nc.vector.tensor_tensor(
                        g_sb[:, :w], g_ps[:, :w], msk_sb[:, :w], mult
                    )
                    g_sbs.append(g_sb)

                if s == 0:
                    if bh == 0:
                        # first bh: the natural-rope chain would gate this
                        # superchunk's m4 through the strict-FIFO DVE queue;
                        # get super-0's natural-layout qr by PE-transposing
                        # qrT instead, and rope only chunks SUP.. on DVE.
                        for ci2 in range(SUP):
                            for m in range(2):
                                t_ps = ps_g.tile([P, P], bf, tag="g", name="t_ps")
                                nc.tensor.transpose(
                                    t_ps[:], qrT[:, m, ds(ci2 * P, P)], ident[:]
                                )
                                nc.vector.tensor_copy(qr[:, m, ci2, :], t_ps[:])
                        emit_nat_rope(SUP)
                    else:
                        emit_nat_rope()

                for ci in range(SUP):
                    i = s * SUP + ci
                    # state += qr_c^T v_c (PSUM accumulate), emitted before the
                    # PV matmuls so the superchunk's last m4 retires early and
                    # the next state cast overlaps the remaining PV work.
                    # Each superchunk's accumulation is a CLOSED group
                    # (stop=True on its last matmul): the state bank is read
                    # (cast) between superchunks, and reading PSUM from an
                    # open accumulation group wedges the device.
                    if 0 < s < NSUP - 1:
                        for m in range(2):
                            for h in range(2):
                                dsl = ds(h * 512, 512)
                                nc.tensor.matmul(
                                    state_ps[:, m, dsl],
                                    qr[:, m, i, :],
                                    v_sb[:, i, dsl],
                                    start=False,
                                    stop=(ci == SUP - 1),
                                    skip_group_check=True,
                                )
                    out_ps = [
                        ps_out.tile([P, 512], f32, tag="outp", name=f"out_ps{h}")
                        for h in range(2)
                    ]
                    first = True
                    if s > 0:
                        # m-outer / h-inner: consecutive matmuls share lhsT
                        for m in range(2):
                            for h in range(2):
                                nc.tensor.matmul(
                                    out_ps[h][:], qrT[:, m, ds(i * P, P)],
                                    state_sb[:, m, ds(h * 512, 512)],
                                    start=(m == 0), stop=False,
                                    skip_group_check=True,
                                )
                        first = False
                    for cj in range(ci + 1):
                        for h in range(2):
                            nc.tensor.matmul(
                                out_ps[h][:],
                                g_sbs[cj][:, ds((ci - cj) * P, P)],
                                v_sb[:, s * SUP + cj, ds(h * 512, 512)],
                                start=first, stop=(cj == ci),
                                skip_group_check=True,
                            )
                        first = False

                    # state += qr_c^T v_c (PSUM accumulate).  Each superchunk's
                    # accumulation is a CLOSED group (stop=True on its last
                    # matmul) because the state bank is read (cast) between
                    # superchunks -- reading PSUM from an open accumulation
                    # group wedges the device.  State after the last
                    # superchunk is never read -> skip those matmuls.
                    out_sb = out_sbs[i // (NCH // 2)]
                    for h in range(2):
                        _copy(
                            nc, OUT_EVAC_ENG[i % 2],
                            out_sb[:, i % (NCH // 2), ds(h * 512, 512)],
                            out_ps[h][:],
                        )
                    if i % SUP == SUP - 1:
                        q0 = (i // SUP) * SUP
                        nc.sync.dma_start(
                            out[bh, :, ds(q0, SUP), :],
                            out_sbs[q0 // (NCH // 2)][:, ds(q0 % (NCH // 2), SUP)],
                        )

                if s == 0:
                    for ci2 in range(SUP):
                        i2 = s * SUP + ci2
                        for m in range(2):
                            for h in range(2):
                                dsl = ds(h * 512, 512)
                                nc.tensor.matmul(
                                    state_ps[:, m, dsl],
                                    qr[:, m, i2, :],
                                    v_sb[:, i2, dsl],
                                    start=(ci2 == 0),
                                    stop=(ci2 == SUP - 1),
                                    skip_group_check=True,
                                )


_BUILT = {}


def _build():
    if "nc" in _BUILT:
        return _BUILT["nc"]
    nc = bacc.Bacc(
        "TRN2", target_bir_lowering=False, debug=False,
        enable_asserts=True, num_devices=NCORES,
    )
    qn = nc.dram_tensor("qn", [HPC, P, 2, NCH, P], bf, kind="ExternalInput")
    qt = nc.dram_tensor("qt", [HPC, 2, P, T], bf, kind="ExternalInput")
    v = nc.dram_tensor("v", [P, NCH, D], bf, kind="ExternalInput")
    cn = nc.dram_tensor("cn", [P, NCH, P], bf, kind="ExternalInput")
    sn = nc.dram_tensor("sn", [P, NCH, P], bf, kind="ExternalInput")
    ct = nc.dram_tensor("ct", [P, T], bf, kind="ExternalInput")
    st = nc.dram_tensor("st", [P, T], bf, kind="ExternalInput")
    mskT = nc.dram_tensor("mskT", [P, SUP * P], bf, kind="ExternalInput")
    out = nc.dram_tensor("out", [HPC, P, NCH, D], bf, kind="ExternalOutput")
    with tile.TileContext(nc) as tc:
        _emit_body(nc, tc, qn, qt, v, cn, sn, ct, st, mskT, out)
    nc.compile()
    _BUILT["nc"] = nc
    return nc


def _host_prep(Q_raw, V_raw):
    """Shard + precompute device inputs (bf16, partition-major layouts)."""
    Q = np.asarray(Q_raw, dtype=np.float32)
    V = np.asarray(V_raw, dtype=np.float32)

    # rope tables, matching reference._get_freqs / _rope in float32
    t = np.arange(N, dtype=np.float32)
    q = np.floor(t / 2.0) * 2.0
    freqs = (1.0 / (THETA ** (q / np.float32(N))) / np.float32(TWO_PI)).astype(
        np.float32
    )
    phases = np.arange(T, dtype=np.float32)[:, None] * freqs[None, :]
    ph = (phases % 1.0) * np.float32(TWO_PI)
    # freqs are equal within each (even, odd) pair -> keep only even columns
    cosf = np.cos(ph[:, 0::2]).astype(bf_np)        # [T, 128]
    sinf = np.sin(ph[:, 0::2]).astype(bf_np)
    # natural tables [P, NCH, P]: (p, c, k) = table[c*128+p, k]
    cn = np.ascontiguousarray(cosf.reshape(NCH, P, P).transpose(1, 0, 2))
    sn = np.ascontiguousarray(sinf.reshape(NCH, P, P).transpose(1, 0, 2))
    # transposed tables [P, T]: (k, t)
    ct = np.ascontiguousarray(cosf.T)
    st = np.ascontiguousarray(sinf.T)
    mskT = np.ones((P, SUP * P), np.float32)
    mskT[:, :P] = np.triu(np.ones((P, P), np.float32), k=1)
    mskT = mskT.astype(bf_np)

    # deinterleave pairs: planes (evens, odds), cast bf16
    Qd = np.stack([Q[..., 0::2], Q[..., 1::2]], axis=2).astype(bf_np)
    # Qd: [B, NH, 2, T, 128]
    # natural layout  [b,h][p, half, c, k] = Qd[b, h, half, c*128+p, k]
    Qn = np.ascontiguousarray(
        Qd.reshape(B, NH, 2, NCH, P, P).transpose(0, 1, 4, 2, 3, 5)
    )  # [B, NH, P, 2, NCH, P]
    # transposed layout [b,h][half, k, t] = Qd[b, h, half, t, k]
    Qt = np.ascontiguousarray(Qd.transpose(0, 1, 2, 4, 3))  # [B, NH, 2, 128, T]

    V16 = V.astype(bf_np)
    # v layout [P, NCH, D]: (p, c, d) = V[c*128+p, d]
    Vp = np.ascontiguousarray(V16.reshape(B, NCH, P, D).transpose(0, 2, 1, 3))

    in_maps = []
    for core in range(NCORES):
        b = core // (NCORES // B)
        hs = (core % (NCORES // B)) * HPC
        in_maps.append(
            {
                "qn": np.ascontiguousarray(Qn[b, hs : hs + HPC]),
                "qt": np.ascontiguousarray(Qt[b, hs : hs + HPC]),
                "v": Vp[b],
                "cn": cn,
                "sn": sn,
                "ct": ct,
                "st": st,
                "mskT": mskT,
            }
        )
    return in_maps


def _run(inputs, trace=False, **kw):
    nc = _build()
    in_maps = _host_prep(inputs["Q_raw"], inputs["V_raw"])
    res = run_bass_kernel_spmd(nc, in_maps, list(range(NCORES)), trace=trace, **kw)
    out = np.empty((B, NH, T, D), dtype=np.float32)
    for core in range(NCORES):
        b = core // (NCORES // B)
        hs = (core % (NCORES // B)) * HPC
        # device out: [HPC, P, NCH, D] partition-major -> [HPC, T, D]
        o = res.results[core]["out"].astype(np.float32)
        out[b, hs : hs + HPC] = o.transpose(0, 2, 1, 3).reshape(HPC, T, D)
    return out, res


def kernel(**inputs):
    out, _ = _run(inputs)
    return out



# revision 3
# speedup vs baseline: 1.0144x; 1.0144x over previous
"""Trainium2 Bass kernel for BDH recurrent (chunked linear) attention.

Problem shapes (hardcoded): Q_raw [2,16,2048,256] f32, V_raw [2,2048,1024] f32,
out [2,16,2048,1024] f32.  8 NeuronCores, data+head parallel: each core owns
4 (batch, head) pairs; V is shared across the 4 heads of a core's batch.

Math (reference semantics), per (b,h), chunks of 128:
  QR = rope(Q); KR = QR
  out_c = q_c @ state_{<c} + (q_c q_c^T  * strict_tril) v_c
  state += q_c^T v_c
Implemented with superchunks of SUP chunks: the recurrent state is accumulated
in PSUM (fp32) across superchunks; within a superchunk the chunk-level
causality is handled with explicit per-pair transposed score blocks
G(j,i) = qr_j qr_i^T (which is scores^T, exactly the lhsT layout the
PV matmul needs; the diagonal block gets the transposed strict-tril mask).

SUP=2 minimizes total PE columns: state ops (q@state / q^T v) cost 2*T*N*D
MACs regardless of SUP, but the triangular G/PV part grows ~linearly with the
superchunk width, so smaller SUP trades a few more state casts for ~12% fewer
matmul columns than SUP=4.

RoPE is computed twice, in the natural [t, n] layout (for the state update's
lhsT) and in the transposed [n, t] layout (for the m1/G lhsT) — the host
supplies Q in both layouts (pair-deinterleaved: (evens | odds), so the
rotation is two plane-wise multiply/adds with no interleave shuffles).
All DRAM layouts are partition-major so every DMA is 128 contiguous
descriptors; the output is written partition-major and un-permuted on host.

Startup: pair 0's transposed q is DMA'd and roped in two column spans so the
first G matmuls issue as soon as the first SUP chunks of data land, instead
of waiting for the full-T rope chain.
"""

import numpy as np
import ml_dtypes

import concourse.mybir as mybir
import concourse.tile as tile
from concourse import bacc
from concourse.bass import ds
from concourse.bass_utils import run_bass_kernel_spmd
from concourse.masks import make_identity

B, NH, T, N, D = 2, 16, 2048, 256, 1024
P = 128          # partition / chunk size
NCH = T // P     # 16 chunks
SUP = 2          # chunks per superchunk
NSUP = NCH // SUP
NQR = NCH - SUP  # natural-layout rope only feeds m4; last sup's m4 is skipped
HPC = 4          # (b,h) pairs per core
NCORES = 8
THETA = 2.0 ** 16
TWO_PI = 2.0 * np.pi

bf = mybir.dt.bfloat16
f32 = mybir.dt.float32
bf_np = ml_dtypes.bfloat16

mult = mybir.AluOpType.mult
add_op = mybir.AluOpType.add
sub_op = mybir.AluOpType.subtract

# engine assignment knobs (tuned from profiles).
# NB: gpsimd tensor_tensor contends with DVE's shared SBUF port (measured 4x
# slowdown on BOTH when concurrent) -> keep all tensor_tensor on DVE.
ROPE_ENG_NAT = "vvvvvv"
ROPE_ENG_TR = "vvvvvv"
# state cast engines for the two [128,1024] tiles
STATE_CAST_ENG = ("v", "s")
# out evacuation engines for the two 512-wide halves of each chunk
OUT_EVAC_ENG = ("s", "v")


def _eng(nc, c):
    return {"g": nc.gpsimd, "v": nc.vector, "s": nc.scalar}[c]


def _copy(nc, c, out, in_):
    if c == "s":
        nc.scalar.copy(out, in_)
    else:
        _eng(nc, c).tensor_copy(out, in_)


def _emit_body(nc, tc, qn, qt, v, cn, sn, ct, st, mskT, out):
    """Tile program for one core: 4 (b,h) pairs, full scan each."""
    with (
        tc.tile_pool(name="const", bufs=1) as constp,
        tc.tile_pool(name="qpool", bufs=2) as qpool,
        tc.tile_pool(name="work", bufs=6) as work,
        tc.tile_pool(name="tmppool", bufs=1) as tmpp,
        tc.tile_pool(name="outbuf", bufs=6) as outp,
        tc.tile_pool(name="statesb", bufs=2) as statep,
        tc.tile_pool(name="ps_state", bufs=1, space="PSUM") as ps_state,
        tc.tile_pool(name="ps_out", bufs=2, space="PSUM") as ps_out,
        tc.tile_pool(name="ps_g", bufs=2, space="PSUM") as ps_g,
    ):
        # resident constants (all DRAM layouts partition-major/contiguous).
        # Load order matters for the startup ramp: the transposed-rope
        # tables gate the first rope TTs -> first; V's first superchunk
        # slice next (first PV), then the natural-rope tables.
        ct_sb = constp.tile([P, T], bf)
        nc.sync.dma_start(ct_sb[:], ct[:, :])
        st_sb = constp.tile([P, T], bf)
        nc.sync.dma_start(st_sb[:], st[:, :])
        msk_sb = constp.tile([P, SUP * P], bf)
        nc.sync.dma_start(msk_sb[:], mskT[:, :])
        ident = constp.tile([P, P], bf)
        make_identity(nc, ident)
        v_sb = constp.tile([P, NCH, D], bf)
        nc.sync.dma_start(v_sb[:, :SUP], v[:, :SUP, :])
        cn_sb = constp.tile([P, NQR, P], bf)
        nc.sync.dma_start(cn_sb[:], cn[:, :NQR, :])
        sn_sb = constp.tile([P, NQR, P], bf)
        nc.sync.dma_start(sn_sb[:], sn[:, :NQR, :])
        nc.sync.dma_start(v_sb[:, SUP:], v[:, SUP:, :])

        for bh in range(HPC):
            qt_sb = qpool.tile([P, 2, T], bf, tag="qt")
            # pair 0 gates the whole startup ramp: land its first SUP
            # chunks first so rope/G can begin while the rest streams.
            spans = [(0, SUP * P), (SUP * P, T - SUP * P)] if bh == 0 else [(0, T)]
            for c0, w in spans:
                nc.scalar.dma_start(qt_sb[:, 0, ds(c0, w)], qt[bh, 0, :, ds(c0, w)])
                nc.scalar.dma_start(qt_sb[:, 1, ds(c0, w)], qt[bh, 1, :, ds(c0, w)])
            qn_sb = qpool.tile([P, 2, NQR, P], bf, tag="qn")
            nc.scalar.dma_start(qn_sb[:], qn[bh, :, :, :NQR])

            # RoPE, transposed layout [n', t] — emitted FIRST: it gates the
            # G/m1 matmuls, and the DVE queue is strict FIFO.
            qrT = qpool.tile([P, 2, T], bf, tag="qrT")
            tmp2 = tmpp.tile([P, T], bf, tag="ropetmp2")
            tmp3 = tmpp.tile([P, T], bf, tag="ropetmp3")
            e = [_eng(nc, c) for c in ROPE_ENG_TR]
            for c0, w in spans:
                sl = ds(c0, w)
                qte, qto = qt_sb[:, 0, sl], qt_sb[:, 1, sl]
                cts, sts = ct_sb[:, sl], st_sb[:, sl]
                r0, r1 = qrT[:, 0, sl], qrT[:, 1, sl]
                t2, t3 = tmp2[:, sl], tmp3[:, sl]
                e[0].tensor_tensor(r0, qte, cts, mult)
                e[1].tensor_tensor(t2, qto, sts, mult)
                e[2].tensor_tensor(r0, r0, t2, sub_op)
                e[3].tensor_tensor(r1, qto, cts, mult)
                e[4].tensor_tensor(t3, qte, sts, mult)
                e[5].tensor_tensor(r1, r1, t3, add_op)

            # RoPE, natural layout (planes are (evens | odds) over pairs):
            #   qr_e = qe*c - qo*s ;  qr_o = qo*c + qe*s
            # Emitted lazily (after the first superchunk's G evacuations):
            # the DVE queue is strict FIFO and qr only gates the m4 state
            # update, so this keeps the first G/PV matmuls unblocked.
            # Only chunks < NQR are roped: the last superchunk's m4 is
            # skipped, so its natural-layout q is never needed.
            qr = qpool.tile([P, 2, NQR, P], bf, tag="qr")

            def emit_nat_rope(c0=0):
                tmp0 = tmpp.tile([P, NQR, P], bf, tag="ropetmp0")
                tmp1 = tmpp.tile([P, NQR, P], bf, tag="ropetmp1")
                e = [_eng(nc, c) for c in ROPE_ENG_NAT]
                qe, qo = qn_sb[:, 0, c0:], qn_sb[:, 1, c0:]
                cns, sns = cn_sb[:, c0:], sn_sb[:, c0:]
                q0, q1 = qr[:, 0, c0:], qr[:, 1, c0:]
                t0_, t1_ = tmp0[:, c0:], tmp1[:, c0:]
                e[0].tensor_tensor(q0, qe, cns, mult)
                e[1].tensor_tensor(t0_, qo, sns, mult)
                e[2].tensor_tensor(q0, q0, t0_, sub_op)
                e[3].tensor_tensor(q1, qo, cns, mult)
                e[4].tensor_tensor(t1_, qe, sns, mult)
                e[5].tensor_tensor(q1, q1, t1_, add_op)

            # chunked scan with PSUM-resident state (fp32, 4 banks)
            state_ps = ps_state.tile([P, 2, D], f32, tag="state")
            for s in range(NSUP):
                if s > 0:
                    state_sb = statep.tile([P, 2, D], bf, tag="state_sb")
                    for m in range(2):
                        for h in range(2):
                            dsl = ds(h * 512, 512)
                            _copy(
                                nc, STATE_CAST_ENG[h],
                                state_sb[:, m, dsl], state_ps[:, m, dsl],
                            )

                # Batched transposed score blocks: for each j-chunk of the
                # superchunk, G_j = qr_j^T-contraction against all i >= j in
                # one matmul (N spans the remaining chunks).  The combined
                # mask (strict-triu block then ones) masks the diagonal
                # block in the same evacuation op.
                g_sbs = []
                for cj in range(SUP):
                    j = s * SUP + cj
                    w = (SUP - cj) * P
                    g_ps = ps_g.tile([P, SUP * P], f32, tag="g", name="g_ps")
                    nc.tensor.matmul(
                        g_ps[:, :w], qrT[:, 0, ds(j * P, P)],
                        qrT[:, 0, ds(j * P, w)], start=True, stop=False,
                    )
                    nc.tensor.matmul(
                        g_ps[:, :w], qrT[:, 1, ds(j * P, P)],
                        qrT[:, 1, ds(j * P, w)], start=False, stop=True,
                    )
                    g_sb = work.tile([P, SUP * P], bf, tag="gsb", name="g_sb")
                    nc.vector.tensor_tensor(
                        g_sb[:, :w], g_ps[:, :w], msk_sb[:, :w], mult
                    )
                    g_sbs.append(g_sb)

                if s == 0:
                    if bh == 0:
                        # first bh: the natural-rope chain would gate this
                        # superchunk's m4 through the strict-FIFO DVE queue;
                        # get super-0's natural-layout qr by PE-transposing
                        # qrT instead, and rope only chunks SUP.. on DVE.
                        for ci2 in range(SUP):
                            for m in range(2):
                                t_ps = ps_g.tile([P, P], bf, tag="g", name="t_ps")
                                nc.tensor.transpose(
                                    t_ps[:], qrT[:, m, ds(ci2 * P, P)], ident[:]
                                )
                                nc.vector.tensor_copy(qr[:, m, ci2, :], t_ps[:])
                        emit_nat_rope(SUP)
                    else:
                        emit_nat_rope()

                for ci in range(SUP):
                    i = s * SUP + ci
                    # state += qr_c^T v_c (PSUM accumulate), emitted before the
                    # PV matmuls so the superchunk's last m4 retires early and
                    # the next state cast overlaps the remaining PV work.
                    # Each superchunk's accumulation is a CLOSED group
                    # (stop=True on its last matmul): the state bank is read
                    # (cast) between superchunks, and reading PSUM from an
                    # open accumulation group wedges the device.
                    if 0 < s < NSUP - 1:
                        for m in range(2):
                            for h in range(2):
                                dsl = ds(h * 512, 512)
                                nc.tensor.matmul(
                                    state_ps[:, m, dsl],
                                    qr[:, m, i, :],
                                    v_sb[:, i, dsl],
                                    start=False,
                                    stop=(ci == SUP - 1),
                                    skip_group_check=True,
                                )
                    out_ps = [
                        ps_out.tile([P, 512], f32, tag="outp", name=f"out_ps{h}")
                        for h in range(2)
                    ]
                    first = True
                    if s > 0:
                        # m-outer / h-inner: consecutive matmuls share lhsT
                        for m in range(2):
                            for h in range(2):
                                nc.tensor.matmul(
                                    out_ps[h][:], qrT[:, m, ds(i * P, P)],
                                    state_sb[:, m, ds(h * 512, 512)],
                                    start=(m == 0), stop=False,
                                    skip_group_check=True,
                                )
                        first = False
                    for cj in range(ci + 1):
                        for h in range(2):
                            nc.tensor.matmul(
                                out_ps[h][:],
                                g_sbs[cj][:, ds((ci - cj) * P, P)],
                                v_sb[:, s * SUP + cj, ds(h * 512, 512)],
                                start=first, stop=(cj == ci),
                                skip_group_check=True,
                            )
                        first = False

                    # evacuate + write back this chunk immediately: the two
                    # 512-halves go to different engines so the chunk's
                    # latency is halved, and the per-chunk DMA keeps the
                    # final writeback small (short kernel tail).
                    out_sb = outp.tile([P, D], bf, tag="out", name="out_sb")
                    for h in range(2):
                        _copy(
                            nc, OUT_EVAC_ENG[h],
                            out_sb[:, ds(h * 512, 512)],
                            out_ps[h][:],
                        )
                    nc.sync.dma_start(out[bh, :, i, :], out_sb[:])

                if s == 0:
                    for ci2 in range(SUP):
                        i2 = s * SUP + ci2
                        for m in range(2):
                            for h in range(2):
                                dsl = ds(h * 512, 512)
                                nc.tensor.matmul(
                                    state_ps[:, m, dsl],
                                    qr[:, m, i2, :],
                                    v_sb[:, i2, dsl],
                                    start=(ci2 == 0),
                                    stop=(ci2 == SUP - 1),
                                    skip_group_check=True,
                                )


_BUILT = {}


def _build():
    if "nc" in _BUILT:
        return _BUILT["nc"]
    nc = bacc.Bacc(
        "TRN2", target_bir_lowering=False, debug=False,
        enable_asserts=True, num_devices=NCORES,
    )
    qn = nc.dram_tensor("qn", [HPC, P, 2, NCH, P], bf, kind="ExternalInput")
    qt = nc.dram_tensor("qt", [HPC, 2, P, T], bf, kind="ExternalInput")
    v = nc.dram_tensor("v", [P, NCH, D], bf, kind="ExternalInput")
    cn = nc.dram_tensor("cn", [P, NCH, P], bf, kind="ExternalInput")
    sn = nc.dram_tensor("sn", [P, NCH, P], bf, kind="ExternalInput")
    ct = nc.dram_tensor("ct", [P, T], bf, kind="ExternalInput")
    st = nc.dram_tensor("st", [P, T], bf, kind="ExternalInput")
    mskT = nc.dram_tensor("mskT", [P, SUP * P], bf, kind="ExternalInput")
    out = nc.dram_tensor("out", [HPC, P, NCH, D], bf, kind="ExternalOutput")
    with tile.TileContext(nc) as tc:
        _emit_body(nc, tc, qn, qt, v, cn, sn, ct, st, mskT, out)
    nc.compile()
    _BUILT["nc"] = nc
    return nc


def _host_prep(Q_raw, V_raw):
    """Shard + precompute device inputs (bf16, partition-major layouts)."""
    Q = np.asarray(Q_raw, dtype=np.float32)
    V = np.asarray(V_raw, dtype=np.float32)

    # rope tables, matching reference._get_freqs / _rope in float32
    t = np.arange(N, dtype=np.float32)
    q = np.floor(t / 2.0) * 2.0
    freqs = (1.0 / (THETA ** (q / np.float32(N))) / np.float32(TWO_PI)).astype(
        np.float32
    )
    phases = np.arange(T, dtype=np.float32)[:, None] * freqs[None, :]
    ph = (phases % 1.0) * np.float32(TWO_PI)
    # freqs are equal within each (even, odd) pair -> keep only even columns
    cosf = np.cos(ph[:, 0::2]).astype(bf_np)        # [T, 128]
    sinf = np.sin(ph[:, 0::2]).astype(bf_np)
    # natural tables [P, NCH, P]: (p, c, k) = table[c*128+p, k]
    cn = np.ascontiguousarray(cosf.reshape(NCH, P, P).transpose(1, 0, 2))
    sn = np.ascontiguousarray(sinf.reshape(NCH, P, P).transpose(1, 0, 2))
    # transposed tables [P, T]: (k, t)
    ct = np.ascontiguousarray(cosf.T)
    st = np.ascontiguousarray(sinf.T)
    mskT = np.ones((P, SUP * P), np.float32)
    mskT[:, :P] = np.triu(np.ones((P, P), np.float32), k=1)
    mskT = mskT.astype(bf_np)

    # deinterleave pairs: planes (evens, odds), cast bf16
    Qd = np.stack([Q[..., 0::2], Q[..., 1::2]], axis=2).astype(bf_np)
    # Qd: [B, NH, 2, T, 128]
    # natural layout  [b,h][p, half, c, k] = Qd[b, h, half, c*128+p, k]
    Qn = np.ascontiguousarray(
        Qd.reshape(B, NH, 2, NCH, P, P).transpose(0, 1, 4, 2, 3, 5)
    )  # [B, NH, P, 2, NCH, P]
    # transposed layout [b,h][half, k, t] = Qd[b, h, half, t, k]
    Qt = np.ascontiguousarray(Qd.transpose(0, 1, 2, 4, 3))  # [B, NH, 2, 128, T]

    V16 = V.astype(bf_np)
    # v layout [P, NCH, D]: (p, c, d) = V[c*128+p, d]
    Vp = np.ascontiguousarray(V16.reshape(B, NCH, P, D).transpose(0, 2, 1, 3))

    in_maps = []
    for core in range(NCORES):
        b = core // (NCORES // B)
        hs = (core % (NCORES // B)) * HPC
        in_maps.append(
            {
                "qn": np.ascontiguousarray(Qn[b, hs : hs + HPC]),
                "qt": np.ascontiguousarray(Qt[b, hs : hs + HPC]),
                "v": Vp[b],
                "cn": cn,
                "sn": sn,
                "ct": ct,
                "st": st,
                "mskT": mskT,
            }
        )
    return in_maps


def _run(inputs, trace=False, **kw):
    nc = _build()
    in_maps = _host_prep(inputs["Q_raw"], inputs["V_raw"])
    res = run_bass_kernel_spmd(nc, in_maps, list(range(NCORES)), trace=trace, **kw)
    out = np.empty((B, NH, T, D), dtype=np.float32)
    for core in range(NCORES):
        b = core // (NCORES // B)
        hs = (core % (NCORES // B)) * HPC
        # device out: [HPC, P, NCH, D] partition-major -> [HPC, T, D]
        o = res.results[core]["out"].astype(np.float32)
        out[b, hs : hs + HPC] = o.transpose(0, 2, 1, 3).reshape(HPC, T, D)
    return out, res


def kernel(**inputs):
    out, _ = _run(inputs)
    return out
